# revision 11
# baseline (speedup 1.0000x reference)
# BitLinear 1.58 (ternary-weight linear with int8 activation quantization)
# on 8 Trainium2 NeuronCores via Bass/Tile.
#
# Reference computation (fp32):
#   w_scale = max(mean(|W|), 1e-5)           (global over the full weight)
#   W_q     = clip(round(W / w_scale), -1, 1)          (ternary)
#   gamma   = max(max(|x|), 1e-5)            (global over the full activation)
#   x_q     = clip(round(x * 128/gamma), -128, 127)
#   out     = (x_q @ W_q^T) * (gamma*w_scale/128) + bias
#
# Sharding: data-parallel over the 8192 tokens (1024 tokens/core), weight
# replicated. Global scales need cross-core reductions; they are shared via
# TWO pipelined 4-byte AllGathers: the w-|sum| partials go first (its local
# reduce needs only the 8 MiB w-slice), so 1/w_scale is ready ~40us before
# gamma and the whole first-column ternarize runs during the gamma
# AllGather's wait window. The x-absmax AllGather follows.
#
# Schedule (v4; see git of v1-v3 in comments of prior revisions):
#  - bias_rep broadcast stages through bias_rep row 0 (DMA + 8 K=1 PE
#    matmuls) with no tile-pool interaction, so the PE queue drains right
#    away and the runtime barrier preceding the collectives clears early.
#  - Stats reads: x as 32 flat [128,1024] chunks (order-independent max),
#    w-slice as the 16-chunk "(a p x) y" view preserved bit-exactly from
#    v1 (the fp32 partial-sum order sets w_scale's last ulp; a 1-ulp shift
#    flips boundary weights at ~100x max-err cost). Chunks round-robin
#    over THREE dma queues (sync/scalar/vector) with 8 outstanding tiles.
#  - Main-loop DMAs dep-gated behind the stats reads; W/x prefetch then
#    floods the AllGather wait windows.
#  - Ternarize: batched [128,2048] tiles (4 k-slices), 3 fused DVE ops;
#    column 0's eight tiles are emitted before the k-loop so they sit
#    ahead of x-quantize in the in-order DVE queue and execute as soon as
#    1/w_scale lands. x-quantize alternates pass1 between ACT and DVE per
#    k-tile to halve the post-gamma ramp.
#  - Token-halved PSUM rotation (4 accumulating + 4 evicting banks), one
#    evict per 8 k-steps: no column-boundary PE bursts.
#
# Quantized operands in bf16 (exact: x_q in [-128,127], W_q in {-1,0,1},
# PSUM accumulates fp32, sums bounded by 4096*128 = 2^19 < 2^24).
# Rounding: round-half-even in fp32 via the magic constant
# (v + 1.5*2^23) - 1.5*2^23, fused into tensor_scalar/activation ops.

import numpy as np
from contextlib import ExitStack

import concourse.bass as bass
import concourse.tile as tile
from concourse import bacc, mybir
from concourse import bass_utils

N_CORES = 8
IN_F = 4096
OUT_F = 4096
TOKENS = 8192  # 4 * 2048
TPC = TOKENS // N_CORES  # tokens per core = 1024
OSL = OUT_F // N_CORES  # per-core weight-stats slice = 512 out_features

KT = IN_F // 128  # 32 k-tiles
KG = KT // 4  # 8 k-groups of 4 (ternarize batch)
CT = OUT_F // 512  # 8 of-columns
TT = TPC // 128  # 8 token-tiles (two halves of 4)

MAGIC = 12582912.0  # 1.5 * 2**23: (v + MAGIC) - MAGIC == round-half-even(v)
EPS = 1e-5
F32 = mybir.dt.float32
BF16 = mybir.dt.bfloat16

NXC = 32  # x-stats chunks [128, 1024]
NWC = 16  # w-stats chunks [128, 1024] (v1 chunking, keeps w_scale bit-exact)

_cache = {}


def _build(dbg=False):
    nc = bacc.Bacc("TRN2", target_bir_lowering=False, debug=False, num_devices=N_CORES)
    xT = nc.dram_tensor("xT", [IN_F, TPC], F32, kind="ExternalInput").ap()
    wT = nc.dram_tensor("wT", [IN_F, OUT_F], F32, kind="ExternalInput").ap()
    wS = nc.dram_tensor("wS", [IN_F, OSL], F32, kind="ExternalInput").ap()
    bias = nc.dram_tensor("bias", [OUT_F], F32, kind="ExternalInput").ap()
    out = nc.dram_tensor("out", [TPC, OUT_F], F32, kind="ExternalOutput").ap()
    if dbg:
        dbg_t = nc.dram_tensor("dbg", [16], F32, kind="ExternalOutput").ap()

    with tile.TileContext(nc) as tc, ExitStack() as ctx:
        ep = ctx.enter_context
        singles = ep(tc.tile_pool(name="singles", bufs=1))
        stream_pool = ep(tc.tile_pool(name="stream", bufs=8))
        win_pool = ep(tc.tile_pool(name="win", bufs=3))
        xin_pool = ep(tc.tile_pool(name="xin", bufs=6))
        xq_pool = ep(tc.tile_pool(name="xq", bufs=KT))
        wq_pool = ep(tc.tile_pool(name="wq", bufs=9))
        ost_pool = ep(tc.tile_pool(name="ost", bufs=3))
        psum_pool = ep(tc.tile_pool(name="psum", bufs=8, space="PSUM"))
        dram = ep(tc.tile_pool(name="dram", bufs=1, space="DRAM"))

        ones_row = singles.tile([1, 128], F32)  # for partition-broadcast matmul
        nc.vector.memset(ones_row[:], 1.0)

        # ---- bias replicated across partitions, FIRST (PE queue drains
        # immediately -> pre-collective barrier clears early). Stages via
        # bias_rep's own row 0: the K=1 matmul reads row 0 of a slice, the
        # copy then overwrites the full slice (row 0 keeps its value).
        bias_rep = singles.tile([128, OUT_F], F32)
        nc.sync.dma_start(bias_rep[0:1, :], bias[:])
        for n in range(CT):
            bp = psum_pool.tile([128, 512], F32, tag="ps", name=f"biasps{n}")
            nc.tensor.matmul(
                bp[:], ones_row[:], bias_rep[0:1, n * 512 : (n + 1) * 512],
                start=True, stop=True,
            )
            nc.scalar.copy(bias_rep[:, n * 512 : (n + 1) * 512], bp[:])

        # ---- stats reads over 3 dma queues ----
        xv = xT[:].rearrange("(p x) y -> p (x y)", p=128)
        wv = wS[:].rearrange("(a p x) y -> a p (x y)", p=128, x=2)
        # DMA-capable engines: the two HWDGE rings + gpsimd's SWDGE as a
        # third stream (higher fixed cost -> give it the smallest share)
        engs = [nc.sync, nc.scalar, nc.gpsimd, nc.sync, nc.scalar]

        xm = singles.tile([128, NXC], F32)
        wm = singles.tile([128, NWC], F32)
        XC = IN_F * TPC // 128 // NXC  # 1024
        last_stats = {}
        for j in range(NWC):
            st = stream_pool.tile([128, 1024], F32, tag="stream", name=f"sw{j}")
            last_stats[j % 2] = engs[j % 2].dma_start(st[:], wv[j])
            nc.scalar.activation(
                st[:], st[:], mybir.ActivationFunctionType.Abs,
                accum_out=wm[:, j : j + 1],
            )
        # w fold + w AllGather (fires ~40us before x stats finish)
        wsumc = singles.tile([128, 1], F32)
        nc.vector.tensor_reduce(
            wsumc[:], wm[:], axis=mybir.AxisListType.X, op=mybir.AluOpType.add
        )
        wsumT = singles.tile([1, 128], F32)
        nc.gpsimd.dma_start(wsumT[:], wsumc[:])
        wsum = singles.tile([1, 1], F32)
        nc.vector.tensor_reduce(
            wsum[:], wsumT[:], axis=mybir.AxisListType.X, op=mybir.AluOpType.add
        )
        ccw_in = dram.tile([1], F32)
        ccw_out = dram.tile([N_CORES], F32)
        nc.gpsimd.dma_start(ccw_in[:], wsum[:])
        nc.gpsimd.collective_compute(
            "AllGather", mybir.AluOpType.bypass,
            replica_groups=[list(range(N_CORES))],
            ins=[ccw_in.opt()], outs=[ccw_out.opt()],
        )

        for j in range(NXC):
            st = stream_pool.tile([128, XC], F32, tag="stream", name=f"sx{j}")
            last_stats[j % 5] = engs[j % 5].dma_start(
                st[:], xv[:, j * XC : (j + 1) * XC]
            )
            nc.vector.tensor_reduce(
                xm[:, j : j + 1], st[:], axis=mybir.AxisListType.X,
                op=mybir.AluOpType.max, apply_absolute_value=True,
            )
        # x fold + x AllGather
        xmax = singles.tile([128, 1], F32)
        nc.vector.tensor_reduce(
            xmax[:], xm[:], axis=mybir.AxisListType.X, op=mybir.AluOpType.max
        )
        xmaxT = singles.tile([1, 128], F32)
        nc.gpsimd.dma_start(xmaxT[:], xmax[:])
        gx = singles.tile([1, 1], F32)
        nc.vector.tensor_reduce(
            gx[:], xmaxT[:], axis=mybir.AxisListType.X, op=mybir.AluOpType.max
        )
        ccx_in = dram.tile([1], F32)
        ccx_out = dram.tile([N_CORES], F32)
        nc.gpsimd.dma_start(ccx_in[:], gx[:])
        nc.gpsimd.collective_compute(
            "AllGather", mybir.AluOpType.bypass,
            replica_groups=[list(range(N_CORES))],
            ins=[ccx_in.opt()], outs=[ccx_out.opt()],
        )
        # gather-result reads, emitted after both doorbells (gpsimd FIFO)
        gw = singles.tile([1, N_CORES], F32)
        nc.gpsimd.dma_start(gw[:], ccw_out[:])
        gxall = singles.tile([1, N_CORES], F32)
        nc.gpsimd.dma_start(gxall[:], ccx_out[:])

        def newton_recip(name, src):
            # correctly-rounded-ish 1/src: HW reciprocal + one Newton step
            r0 = singles.tile([1, 1], F32, tag=f"{name}r0")
            nc.vector.reciprocal(r0[:], src[:])
            t = singles.tile([1, 1], F32, tag=f"{name}t")
            nc.vector.tensor_tensor(t[:], src[:], r0[:], op=mybir.AluOpType.mult)
            u = singles.tile([1, 1], F32, tag=f"{name}u")
            nc.vector.tensor_scalar(
                u[:], t[:], -1.0, 2.0, mybir.AluOpType.mult, mybir.AluOpType.add
            )
            r1 = singles.tile([1, 1], F32, tag=f"{name}r1")
            nc.vector.tensor_tensor(r1[:], r0[:], u[:], op=mybir.AluOpType.mult)
            return r1

        # ---- w-scale path (unblocks on the w AllGather) ----
        gsum = singles.tile([1, 1], F32)
        nc.vector.tensor_reduce(
            gsum[:], gw[:], axis=mybir.AxisListType.X, op=mybir.AluOpType.add
        )
        wscale = singles.tile([1, 1], F32)
        nc.vector.tensor_scalar(
            wscale[:], gsum[:], 1.0 / (OUT_F * IN_F), EPS,
            mybir.AluOpType.mult, mybir.AluOpType.max,
        )
        rw = newton_recip("rw", wscale)  # 1/w_scale
        bprw = psum_pool.tile([128, 1], F32, tag="ps", name="bprw")
        nc.tensor.matmul(bprw[:], ones_row[:], rw[:], start=True, stop=True)
        b_rw = singles.tile([128, 1], F32)
        nc.vector.tensor_copy(b_rw[:], bprw[:])
        r_w = b_rw[:, 0:1]

        # ---- gamma path (unblocks on the x AllGather) ----
        gmax = singles.tile([1, 1], F32)
        nc.vector.tensor_reduce(
            gmax[:], gxall[:], axis=mybir.AxisListType.X, op=mybir.AluOpType.max
        )
        gamma = singles.tile([1, 1], F32)
        nc.vector.tensor_scalar(gamma[:], gmax[:], EPS, None, mybir.AluOpType.max)
        rg = newton_recip("rg", gamma)  # 1/gamma
        pack2 = singles.tile([1, 2], F32)
        nc.vector.tensor_scalar(
            pack2[0:1, 0:1], rg[:], 128.0, None, mybir.AluOpType.mult
        )
        gws = singles.tile([1, 1], F32)
        nc.vector.tensor_tensor(gws[:], gamma[:], wscale[:], op=mybir.AluOpType.mult)
        nc.vector.tensor_scalar(
            pack2[0:1, 1:2], gws[:], 2.0 ** -7, None, mybir.AluOpType.mult
        )
        bp2 = psum_pool.tile([128, 2], F32, tag="ps", name="bp2")
        nc.tensor.matmul(bp2[:], ones_row[:], pack2[:], start=True, stop=True)
        b2 = singles.tile([128, 2], F32)
        nc.vector.tensor_copy(b2[:], bp2[:])
        s_x = b2[:, 0:1]
        s_o = b2[:, 1:2]

        if dbg:
            dsb = singles.tile([1, 16], F32)
            nc.vector.memset(dsb[:], 0.0)
            nc.vector.tensor_copy(dsb[0:1, 0:1], gamma[:])
            nc.vector.tensor_copy(dsb[0:1, 1:2], wscale[:])
            nc.vector.tensor_copy(dsb[0:1, 2:3], b2[96:97, 0:1])
            nc.vector.tensor_copy(dsb[0:1, 3:4], b_rw[96:97, :])
            nc.sync.dma_start(dbg_t[:], dsb[:])

        # ---- main loop ----
        xq = [None] * KT

        def emit_xq(k):
            # x requantize read; all rings, first ones gated behind stats
            # sync/scalar only: a vector-issued DMA trigger would queue
            # behind quantize ops in the in-order DVE stream
            xin = xin_pool.tile([128, TPC], F32, tag="xin", name=f"xin_q{k}")
            eng = nc.sync if k % 2 == 0 else nc.scalar
            xin_dma = eng.dma_start(xin[:], xT[k * 128 : (k + 1) * 128, :])
            if k < 6:
                for ring in last_stats:
                    tile.add_dep_helper(
                        xin_dma.ins, last_stats[ring].ins, sync=True,
                        reason="hold x re-read until stats reads finish",
                    )
            # pass1: t = x*s_x + MAGIC (rounds to int); alternate ACT/DVE per
            # k to halve the post-gamma ramp. pass2 (DVE): min(t-M, 127)
            if k % 2 == 0:
                nc.scalar.activation(
                    xin[:], xin[:], mybir.ActivationFunctionType.Copy,
                    scale=s_x, bias=MAGIC,
                )
            else:
                nc.vector.tensor_scalar(
                    xin[:], xin[:], s_x, MAGIC, mybir.AluOpType.mult,
                    mybir.AluOpType.add,
                )
            xq_k = xq_pool.tile([128, TPC], BF16, tag="xq", name=f"xq{k}")
            nc.vector.tensor_scalar(
                xq_k[:], xin[:], MAGIC, 127.0, mybir.AluOpType.subtract,
                mybir.AluOpType.min,
            )
            xq[k] = xq_k

        def emit_wq(c, g):
            # one DMA brings 4 k-slices [128, 2048]; 3 fused DVE passes:
            # t = w*r_w + MAGIC; clip to MAGIC+-1; -MAGIC (cast bf16)
            win = win_pool.tile([128, 2048], F32, tag="win", name=f"win_c{c}_g{g}")
            src = wT[g * 512 : (g + 1) * 512, c * 512 : (c + 1) * 512]
            eng = nc.sync if g % 2 == 0 else nc.scalar
            win_dma = eng.dma_start(
                win[:].rearrange("p (x y) -> p x y", y=512),
                src.rearrange("(x p) y -> p x y", p=128),
            )
            if c == 0 and g < 3:
                for ring in last_stats:
                    tile.add_dep_helper(
                        win_dma.ins, last_stats[ring].ins, sync=True,
                        reason="hold weight prefetch until stats reads finish",
                    )
            nc.vector.tensor_scalar(
                win[:], win[:], r_w, MAGIC, mybir.AluOpType.mult,
                mybir.AluOpType.add,
            )
            nc.vector.tensor_scalar(
                win[:], win[:], MAGIC + 1.0, MAGIC - 1.0, mybir.AluOpType.min,
                mybir.AluOpType.max,
            )
            wq = wq_pool.tile([128, 2048], BF16, tag="wq", name=f"wq_c{c}_g{g}")
            nc.vector.tensor_scalar(
                wq[:], win[:], MAGIC, None, mybir.AluOpType.subtract
            )
            return wq

        def emit_evict(c, t, psum_t):
            of = c * 512
            osb = ost_pool.tile([128, 512], F32, tag="ost", name=f"osb_c{c}_t{t}")
            # out = psum * s_o + bias, one DVE op straight from PSUM
            nc.vector.scalar_tensor_tensor(
                osb[:], psum_t[:], s_o, bias_rep[:, of : of + 512],
                op0=mybir.AluOpType.mult, op1=mybir.AluOpType.add,
            )
            eng = nc.scalar if t % 2 == 0 else nc.sync
            eng.dma_start(out[t * 128 : (t + 1) * 128, of : of + 512], osb[:])

        prev = None  # (c, half_t0, psums) awaiting evict
        for c in range(CT):
            wqs = [None] * KG
            if c == 0:
                # pre-queue the whole first column's ternarize so it sits
                # ahead of x-quantize in the DVE queue and runs as soon as
                # 1/w_scale lands (during the gamma AllGather wait)
                for g in range(KG):
                    wqs[g] = emit_wq(0, g)
            for half in range(2):
                t0 = half * 4
                psums = [
                    psum_pool.tile(
                        [128, 512], F32, tag="ps", name=f"ps_c{c}_t{t0 + i}"
                    )
                    for i in range(4)
                ]
                for k in range(KT):
                    if c == 0 and half == 0:
                        emit_xq(k)
                    if c > 0 and half == 0 and k % 4 == 0:
                        wqs[k // 4] = emit_wq(c, k // 4)
                    # previous half's evicts, one per 8 k-steps: banks free
                    # gradually without a DVE burst
                    if prev is not None and k % 8 == 4:
                        pc, pt0, pp = prev
                        i = (k - 4) // 8
                        emit_evict(pc, pt0 + i, pp[i])
                    wq_s = wqs[k // 4][:, (k % 4) * 512 : (k % 4 + 1) * 512]
                    for i in range(4):
                        t = t0 + i
                        nc.tensor.matmul(
                            psums[i][:], xq[k][:, t * 128 : (t + 1) * 128], wq_s,
                            start=(k == 0), stop=(k == KT - 1),
                        )
                prev = (c, t0, psums)
        pc, pt0, pp = prev
        for i in range(4):
            emit_evict(pc, pt0 + i, pp[i])

    nc.compile()
    return nc


def _prep_inputs(x, weight, bias):
    x2 = np.ascontiguousarray(x.reshape(TOKENS, IN_F).T)  # [IN_F, TOKENS]
    wT = np.ascontiguousarray(weight.T)  # [IN_F, OUT_F]
    in_maps = []
    for i in range(N_CORES):
        in_maps.append(
            {
                "xT": np.ascontiguousarray(x2[:, i * TPC : (i + 1) * TPC]),
                "wT": wT,
                "wS": np.ascontiguousarray(wT[:, i * OSL : (i + 1) * OSL]),
                "bias": bias,
            }
        )
    return in_maps


def _run(x, weight, bias, trace=False):
    if "nc" not in _cache:
        _cache["nc"] = _build()
    nc = _cache["nc"]
    in_maps = _prep_inputs(
        np.asarray(x, dtype=np.float32),
        np.asarray(weight, dtype=np.float32),
        np.asarray(bias, dtype=np.float32),
    )
    res = bass_utils.run_bass_kernel_spmd(
        nc, in_maps, list(range(N_CORES)), trace=trace
    )
    full = np.concatenate(
        [res.results[i]["out"] for i in range(N_CORES)], axis=0
    )
    return full.reshape(4, 2048, OUT_F), res


def kernel(x, weight, bias):
    out, _ = _run(x, weight, bias)
    return out
